# revision 3
# baseline (speedup 1.0000x reference)
"""Trainium2 Bass kernel for a custom GRU (nn_BasicGRU).

Reference computation (per batch row b, h0 = 0):
    for t in 0..T-1:
        comb  = [x_t, h]                          # [I+H]
        z     = sigmoid(comb @ Wz + bz)
        r     = sigmoid(comb @ Wr + br)
        comb2 = [x_t, r*h]
        hc    = tanh(comb2 @ Wh + bh)
        h     = (1-z)*h + z*hc
        y_t   = h

Shapes: x [128, 1024, 256] f32, W* [768, 512] f32, b* [512] f32,
y [128, 1024, 512] f32.

Strategy (8 NeuronCores, data-parallel over batch, 16 rows/core):
- All on-chip state kept "H-major": feature dim on partitions, batch in the
  free dim.  State h is [128 partitions, 4 chunks, 16 batch] (feature
  f = chunk*128 + partition).  This makes every per-step elementwise op a
  cheap [128, 64] op and avoids any transposes in the recurrence.
- Weights are the matmul stationary operand (lhsT = W[kchunk, mchunk] tile),
  the streamed operand is the small h / (r*h) tile [128, 16].  With a
  16-wide moving stream the per-step cost is dominated by LDWEIGHTS
  (128 cols/tile, FWL rate: 2 bf16 or 4 fp8 per cycle @1.2GHz), so the
  z/r gate weights (x-part and h-part) are quantized to fp8 e3m4 with a
  power-of-2 scale (x256, folded back via the sigmoid's scale=1/256) --
  halving their weight-load cost.  The tanh path (Wh) is precision
  critical and stays bf16 (fp32 PSUM accumulation everywhere).
- The x-dependent part of all three gate pre-activations (x_t @ Wx_g + b_g)
  does not depend on the recurrence: it is computed by efficient batched
  matmuls directly into the same PSUM banks the recurrent matmuls then
  accumulate into (one bank per gate per 8-step block).
- The master h state stays fp32, with a bf16 copy made each step for the
  next step's matmuls.
- Output y is written H-major to HBM and rearranged to [B, T, H] on host.
"""

import numpy as np
import ml_dtypes

import concourse.bass as bass
import concourse.tile as tile
from concourse import bacc, mybir
from concourse.bass_utils import run_bass_kernel_spmd

F32 = mybir.dt.float32
BF16 = mybir.dt.bfloat16
FP8 = mybir.dt.float8e3

N_CORES = 8
B = 128
I_DIM = 256
H_DIM = 512
B_LOC = B // N_CORES          # 16 batch rows per core
BLK = 8                       # recurrence steps per PSUM block
KC = H_DIM // 128             # 4 k-chunks for the h-part contraction
MC = H_DIM // 128             # 4 output-feature chunks
IC = I_DIM // 128             # 2 k-chunks for the x-part contraction
GZ, GR, GH = 0, 1, 2          # gate order in the stacked weight tensors
W_SCALE = 256.0               # fp8 weight scale for the z/r gates
INV_SCALE = 1.0 / W_SCALE

_CACHE = {}


def build_gru_nc(T, repeat=1, loop_blocks=0):
    """Build the Bass/Tile program for a T-step GRU on one core.

    repeat>1 wraps the whole recurrence in a hardware For_i loop that
    re-runs it from h0 (used only for on-device timing via the R-slope).
    loop_blocks=LB>0 emits a hardware For_i loop whose body processes LB
    8-step blocks (keeps the instruction stream small: long fully
    unrolled streams measure ~25% slower per step due to instruction
    fetch).  State is carried across iterations in dedicated tiles."""
    NB = T // BLK
    assert T % BLK == 0

    nc = bacc.Bacc("TRN2", target_bir_lowering=False, debug=False,
                   enable_asserts=False, num_devices=N_CORES)

    xT_cols = T * B_LOC + (loop_blocks * BLK * B_LOC if loop_blocks else 0)
    xT = nc.dram_tensor("xT", [IC, 128, xT_cols], BF16, kind="ExternalInput")
    whh8 = nc.dram_tensor("whh8", [128, 2, KC, MC, 128], FP8,
                          kind="ExternalInput")
    whhh = nc.dram_tensor("whhh", [128, KC, MC, 128], BF16,
                          kind="ExternalInput")
    wx8 = nc.dram_tensor("wx8", [128, 2, IC, MC, 128], FP8,
                         kind="ExternalInput")
    wxh = nc.dram_tensor("wxh", [128, IC, MC, 128], BF16,
                         kind="ExternalInput")
    bias = nc.dram_tensor("bias", [1, 3, MC, 128], BF16, kind="ExternalInput")
    y = nc.dram_tensor("y", [128, MC, T * B_LOC], F32, kind="ExternalOutput")

    with tile.TileContext(nc) as tc:
        with (
            tc.tile_pool(name="const", bufs=1) as const,
            tc.tile_pool(name="xp", bufs=4) as xp,
            tc.tile_pool(name="yp", bufs=3) as yp,
            tc.tile_pool(name="hp", bufs=4) as hp,
            tc.tile_pool(name="sp", bufs=4) as sp,
            tc.tile_pool(name="ps", bufs=2, space="PSUM") as ps,
        ):
            # ---- constants ----
            whh8_s = const.tile([128, 2, KC, MC, 128], FP8, tag="whh8")
            nc.sync.dma_start(whh8_s[:], whh8[:])
            whhh_s = const.tile([128, KC, MC, 128], BF16, tag="whhh")
            nc.sync.dma_start(whhh_s[:], whhh[:])
            wx8_s = const.tile([128, 2, IC, MC, 128], FP8, tag="wx8")
            nc.sync.dma_start(wx8_s[:], wx8[:])
            wxh_s = const.tile([128, IC, MC, 128], BF16, tag="wxh")
            nc.sync.dma_start(wxh_s[:], wxh[:])
            bias_s = const.tile([1, 3, MC, 128], BF16, tag="bias")
            nc.sync.dma_start(bias_s[:], bias[:])
            ones_s = const.tile([1, BLK * B_LOC], BF16, tag="ones")
            nc.vector.memset(ones_s[:], 1.0)
            h0_b = const.tile([128, KC, B_LOC], BF16, tag="h0b")
            nc.vector.memset(h0_b[:], 0.0)
            h0_f = const.tile([128, KC, B_LOC], F32, tag="h0f")
            nc.vector.memset(h0_f[:], 0.0)

            def xpart_work(psg, xt_full, ones_ap):
                """x-part matmul work list for one 8-step block: for each
                gate/mc, 2 k-chunk matmuls off the streamed x tile plus the
                bias via a ones-row matmul (z/r in fp8, h in bf16)."""
                work = []
                for g in range(3):
                    for mc in range(MC):
                        out_ap = psg[g][:, mc, :]
                        for ic in range(IC):
                            if g == GH:
                                lhsT = wxh_s[:, ic, mc, :]
                            else:
                                lhsT = wx8_s[:, g, ic, mc, :]
                            work.append((out_ap, lhsT, xt_full[:, ic, :],
                                         mc == 0 and ic == 0))
                        work.append((out_ap, bias_s[:, g, mc, :], ones_ap,
                                     False))
                return work

            def emit_xpre(work, n):
                for _ in range(min(n, len(work))):
                    out_ap, lhsT, rhs, is_start = work.pop(0)
                    nc.tensor.matmul(out_ap, lhsT, rhs,
                                     start=is_start, stop=False)

            def emit_step(psg, tl, h_prev_b, h_prev_f, ysv, xwork, h_b_out):
                """One recurrence step; returns (h_b, h_new_f) APs."""
                s0 = tl * B_LOC
                s1 = (tl + 1) * B_LOC
                # r-gate h-part matmuls (fp8 weights; critical path)
                for mc in range(MC):
                    for kc in range(KC):
                        nc.tensor.matmul(
                            psg[GR][:, mc, s0:s1],
                            whh8_s[:, GR, kc, mc, :],
                            h_prev_b[:, kc, :],
                            start=False, stop=(kc == KC - 1),
                        )
                r_b = sp.tile([128, MC, B_LOC], BF16, tag="r_b", name="r_b")
                nc.scalar.activation(
                    r_b[:], psg[GR][:, :, s0:s1],
                    func=mybir.ActivationFunctionType.Sigmoid,
                    scale=INV_SCALE)
                rh_b = sp.tile([128, MC, B_LOC], BF16, tag="rh_b",
                               name="rh_b")
                nc.vector.tensor_mul(rh_b[:], r_b[:], h_prev_b)
                emit_xpre(xwork, 5)
                # candidate-gate h-part matmuls (bf16 weights; critical)
                for mc in range(MC):
                    for kc in range(KC):
                        nc.tensor.matmul(
                            psg[GH][:, mc, s0:s1],
                            whhh_s[:, kc, mc, :],
                            rh_b[:, kc, :],
                            start=False, stop=(kc == KC - 1),
                        )
                # z-gate h-part matmuls (fp8; off the critical path -- its
                # result is only needed at the blend)
                for mc in range(MC):
                    for kc in range(KC):
                        nc.tensor.matmul(
                            psg[GZ][:, mc, s0:s1],
                            whh8_s[:, GZ, kc, mc, :],
                            h_prev_b[:, kc, :],
                            start=False, stop=(kc == KC - 1),
                        )
                hc_f = sp.tile([128, MC, B_LOC], F32, tag="hc_f", name="hc_f")
                nc.scalar.activation(
                    hc_f[:], psg[GH][:, :, s0:s1],
                    func=mybir.ActivationFunctionType.Tanh)
                z_b = sp.tile([128, MC, B_LOC], BF16, tag="z_b", name="z_b")
                nc.scalar.activation(
                    z_b[:], psg[GZ][:, :, s0:s1],
                    func=mybir.ActivationFunctionType.Sigmoid,
                    scale=INV_SCALE)
                negb_f = sp.tile([128, MC, B_LOC], F32, tag="negb_f",
                                 name="negb_f")
                nc.vector.scalar_tensor_tensor(
                    negb_f[:], z_b[:], 1.0, h_prev_f,
                    op0=mybir.AluOpType.subtract,
                    op1=mybir.AluOpType.mult)
                a_f = sp.tile([128, MC, B_LOC], F32, tag="a_f", name="a_f")
                nc.vector.tensor_mul(a_f[:], z_b[:], hc_f[:])
                # blend h' = z*hc - (z-1)*h: bf16 copy (feeds next step's
                # matmuls) on DVE; fp32 master into the y staging tile on
                # the Pool engine, off the critical path.
                if h_b_out is None:
                    h_b_out = hp.tile([128, KC, B_LOC], BF16, tag="h_b",
                                      name="h_b")
                nc.vector.tensor_sub(h_b_out[:], a_f[:], negb_f[:])
                h_new_f = ysv[:, :, tl, :]
                nc.gpsimd.tensor_sub(h_new_f, a_f[:], negb_f[:])
                return h_b_out[:], h_new_f

            def emit_body():
                h_prev_b = h0_b[:]
                h_prev_f = h0_f[:]
                w = BLK * B_LOC

                def alloc_block(c0):
                    xt = xp.tile([128, IC, w], BF16, tag="xt", name="xt")
                    nc.sync.dma_start(
                        xt[:], xT[:, :, bass.ds(c0, w)].rearrange(
                            "i p n -> p i n"))
                    psg = [ps.tile([128, MC, w], F32, tag=f"ps{g}",
                                   name=f"ps{g}") for g in range(3)]
                    return psg, xpart_work(psg, xt[:], ones_s[:])

                psg, xwork = alloc_block(0)
                emit_xpre(xwork, len(xwork))

                for blk in range(NB):
                    c0 = blk * w
                    ys = yp.tile([128, MC, w], F32, tag="ys")
                    ysv = ys.rearrange("p m (t b) -> p m t b", b=B_LOC)
                    if blk + 1 < NB:
                        psg_next, xwork = alloc_block((blk + 1) * w)
                    else:
                        psg_next, xwork = None, []
                    for tl in range(BLK):
                        h_prev_b, h_prev_f = emit_step(
                            psg, tl, h_prev_b, h_prev_f, ysv, xwork, None)
                    nc.gpsimd.dma_start(y[:, :, bass.ds(c0, w)], ys[:])
                    psg = psg_next

            def emit_loop():
                LB = loop_blocks
                assert NB % LB == 0 and LB % 2 == 0
                n_iters = NB // LB
                w = BLK * B_LOC

                carry_b = const.tile([128, KC, B_LOC], BF16, tag="carryb")
                nc.vector.memset(carry_b[:], 0.0)
                carry_f = const.tile([128, KC, B_LOC], F32, tag="carryf")
                nc.vector.memset(carry_f[:], 0.0)
                # fixed double-buffered cross-edge tiles (explicit parity so
                # addresses line up across the loop back-edge)
                xt_bufs = [const.tile([128, IC, w], BF16, tag=f"xtb{i}",
                                      name=f"xtb{i}") for i in range(2)]
                ps_bufs = [[ps.tile([128, MC, w], F32, tag=f"ps{g}b{i}",
                                    name=f"ps{g}b{i}", bufs=1)
                            for i in range(2)] for g in range(3)]

                def alloc_block_fixed(c0, parity):
                    xt = xt_bufs[parity]
                    nc.sync.dma_start(
                        xt[:], xT[:, :, bass.ds(c0, w)].rearrange(
                            "i p n -> p i n"))
                    psg = [ps_bufs[g][parity] for g in range(3)]
                    return psg, xpart_work(psg, xt[:], ones_s[:])

                psg, xwork = alloc_block_fixed(0, 0)
                emit_xpre(xwork, len(xwork))

                with tc.For_i(0, n_iters, 1) as iv:
                    base = iv * (LB * w)
                    h_prev_b = carry_b[:]
                    h_prev_f = carry_f[:]
                    for bi in range(LB):
                        c0 = base + bi * w
                        ys = yp.tile([128, MC, w], F32, tag="ys", name="ys")
                        ysv = ys.rearrange("p m (t b) -> p m t b", b=B_LOC)
                        psg_next, xwork = alloc_block_fixed(
                            base + (bi + 1) * w, (bi + 1) % 2)
                        for tl in range(BLK):
                            last = (bi == LB - 1 and tl == BLK - 1)
                            h_prev_b, h_new_f = emit_step(
                                psg, tl, h_prev_b, h_prev_f, ysv, xwork,
                                carry_b if last else None)
                            if last:
                                nc.vector.tensor_copy(carry_f[:], h_new_f)
                                h_prev_f = carry_f[:]
                            else:
                                h_prev_f = h_new_f
                        nc.gpsimd.dma_start(y[:, :, bass.ds(c0, w)], ys[:])
                        psg = psg_next

            if loop_blocks:
                emit_loop()
            elif repeat == 1:
                emit_body()
            else:
                with tc.For_i(0, repeat, 1):
                    emit_body()

    nc.finalize()
    return nc


def _q8(a):
    """Scale, clip to the TRN e3m4 normal range, quantize."""
    return np.clip(a * W_SCALE, -15.5, 15.5).astype(ml_dtypes.float8_e3m4)


def _host_prep_weights(Wz, bz, Wr, br, Wh, bh):
    Wzr = np.stack([Wz, Wr])                         # [2, 768, 512]
    wx8_host = np.ascontiguousarray(
        _q8(Wzr[:, :I_DIM, :]).reshape(2, IC, 128, MC, 128)
        .transpose(2, 0, 1, 3, 4))                   # [128, 2, IC, MC, 128]
    whh8_host = np.ascontiguousarray(
        _q8(Wzr[:, I_DIM:, :]).reshape(2, KC, 128, MC, 128)
        .transpose(2, 0, 1, 3, 4))                   # [128, 2, KC, MC, 128]
    wxh_host = np.ascontiguousarray(
        Wh[:I_DIM, :].reshape(IC, 128, MC, 128).transpose(1, 0, 2, 3)
    ).astype(ml_dtypes.bfloat16)                     # [128, IC, MC, 128]
    whhh_host = np.ascontiguousarray(
        Wh[I_DIM:, :].reshape(KC, 128, MC, 128).transpose(1, 0, 2, 3)
    ).astype(ml_dtypes.bfloat16)                     # [128, KC, MC, 128]
    bias_host = np.stack([bz * W_SCALE, br * W_SCALE, bh]).reshape(
        1, 3, MC, 128).astype(ml_dtypes.bfloat16)
    return wx8_host, whh8_host, wxh_host, whhh_host, bias_host


def make_in_maps(x, Wz, bz, Wr, br, Wh, bh, pad_blocks=0):
    x = np.asarray(x)
    T = x.shape[1]
    assert x.shape == (B, T, I_DIM)
    wx8_host, whh8_host, wxh_host, whhh_host, bias_host = _host_prep_weights(
        np.asarray(Wz), np.asarray(bz), np.asarray(Wr), np.asarray(br),
        np.asarray(Wh), np.asarray(bh))
    in_maps = []
    for c in range(N_CORES):
        xc = x[c * B_LOC:(c + 1) * B_LOC]            # [16, T, 256]
        xTc = np.ascontiguousarray(xc.transpose(2, 1, 0)).reshape(
            IC, 128, T * B_LOC).astype(ml_dtypes.bfloat16)
        if pad_blocks:
            pad = np.zeros((IC, 128, pad_blocks * BLK * B_LOC),
                           ml_dtypes.bfloat16)
            xTc = np.concatenate([xTc, pad], axis=2)
        in_maps.append({
            "xT": xTc,
            "whh8": whh8_host,
            "whhh": whhh_host,
            "wx8": wx8_host,
            "wxh": wxh_host,
            "bias": bias_host,
        })
    return in_maps


def assemble_output(y_cat, T):
    """y_cat: [N_CORES*128, MC, T*B_LOC] (concatenated per-core 'y' outputs)
    -> [B, T, H]."""
    y_cat = np.asarray(y_cat).reshape(N_CORES, 128, MC, T, B_LOC)
    out = y_cat.transpose(0, 4, 3, 2, 1).reshape(B, T, H_DIM)
    return np.ascontiguousarray(out, dtype=np.float32)


def kernel(x, Wz, bz, Wr, br, Wh, bh):
    x = np.asarray(x)
    T = x.shape[1]
    # hardware For_i variant when the block structure allows it (identical
    # numerics, far faster to compile); fully unrolled fallback otherwise
    NBt = T // BLK
    lb = next((n for n in (32, 16, 8) if NBt % n == 0), 0)
    in_maps = make_in_maps(x, Wz, bz, Wr, br, Wh, bh, pad_blocks=lb)

    key = (T, lb)
    if key not in _CACHE:
        _CACHE[key] = build_gru_nc(T, loop_blocks=lb)
    nc = _CACHE[key]

    res = run_bass_kernel_spmd(nc, in_maps, core_ids=list(range(N_CORES)))
    y_cat = np.concatenate([res.results[c]["y"] for c in range(N_CORES)], axis=0)
    return assemble_output(y_cat, T)
